# revision 3
# baseline (speedup 1.0000x reference)
"""Trainium2 Bass kernel for NnqlmCnnBasedLstm.

Math (per batch item, per input sequence q/a):
  xe = embed[idx]                      (L, D)       D = 128
  dens_t = outer(xe_t, xe_t)/(|xe_t|^2 + 1e-4)     (D, D), symmetric
  2-layer ConvLSTM over L=40 steps; each gate g:
    pre_g = conv2d([xt; h], W_g, stride=(2,1), pad=(1,1)) + b_g  on (2D, D) -> (D, D)
  c = sig(f)*c + ig*tanh(cc); h = og*tanh(c)
  out = max_t h2_t  -> flatten -> concat(q,a) -> linear(2) -> log_softmax

Device strategy (8 cores, data parallel over B=32 -> 4 items/core, each with a
q-chain and an a-chain = 8 chains/core):
  * State kept TRANSPOSED: tiles are (w partitions, j free).  Densities are
    symmetric, precomputed on HOST, and DMAed per step (DMA engines are idle).
  * conv: out_T[w, j] = sum_{dh,dw} W[dh,dw] * inp_T[w-1+dw, 2j-1+dh].
    For each dh this is a 3-diagonal Toeplitz band matrix (over w) applied on
    the TensorEngine.  fp8e4m3 + perf_mode=DoubleRow packs the (dh0,dh1) and
    (dh2,dh3) band pairs as two K-planes of one matmul (the plane pairs are
    adjacent columns of the input tile), halving the matmul count; fp32 PSUM
    accumulates the two pair-matmuls per gate.  All 8 chains batched in the
    moving free dim (2 x 512-col halves per PSUM bank limit).
  * The two ConvLSTM layers are software-pipelined with a 1-step skew
    (emit P0(t) then P1(t-1)): the PE runs one layer's matmul burst while the
    other layer's Activation/Vector chain completes.
  * Engine balance (ScalarE/ACT was the 94%-busy bottleneck): f/i/cs/th stay
    on ScalarE reading PSUM -> bf16.  The OG gate's sigmoid is replaced by an
    exact-clamped piecewise-quadratic sig(x) ~ 0.5 + 0.5*z*(2-|z|),
    z = clamp(s*x + s*b, -1, 1), computed jointly on DVE (PSUM clamp-hi,
    |a| STT, final STT -- GPSIMD can't touch PSUM or run TT/STT) and
    GpSimd/Pool (the two tensor_scalar steps).  OG conv weights/bias are
    pre-scaled by s on the host so PSUM already holds s*pre.  The +0.5 folds
    into the h = sig(o)*th multiply via scalar_tensor_tensor.  A few chains
    of layer-0's og stay on ScalarE (OG0_ACT_CH) to balance ACT vs DVE.
  * Cell updates on VectorE in bf16 (2x perf mode); h is produced in bf16 and
    cast to fp8 for the next-step conv input on GpSimd (tensor_copy), which
    was idle; the running max-pool stays on VectorE.
  * Embedding gather, final linear + log_softmax on host (tiny).
"""

import os
import sys

import numpy as np

for _p in ("/opt/trn_rl_repo", "/root/.axon_site/_ro/trn_rl_repo"):
    if os.path.isdir(_p) and _p not in sys.path:
        sys.path.insert(0, _p)

B, L, D, V, NL = 32, 40, 128, 32000, 2
NCORES = 8
CH = 8            # chains per core: 4 batch items x {q, a}
SEG = 2 * D + 2   # per-chain column span in the input tile: [0]=0, [1..128]=x, [129..256]=h, [257]=0
NF = CH * SEG
S_OG = 0.22       # og-gate pre-activation scale for the quadratic sigmoid
OG0_ACT_CH = 3    # layer-0 og: chains kept on ScalarE (rest use the approx)
OG1_ACT_CH = 0    # layer-1 og: chains kept on ScalarE

_CACHE = {}


def _build_nc(L=L):
    import concourse.bass as bass
    import concourse.bacc as bacc
    import concourse.mybir as mybir
    from concourse import tile

    f32 = mybir.dt.float32
    bf16 = mybir.dt.bfloat16
    fp8 = mybir.dt.float8e4
    i8 = mybir.dt.int8
    AF = mybir.ActivationFunctionType
    ALU = mybir.AluOpType
    DR = mybir.MatmulPerfMode.DoubleRow

    nc = bacc.Bacc(None, target_bir_lowering=False)

    dens_d = nc.dram_tensor("dens", (L, D, CH * D), fp8, kind="ExternalInput")
    st_d = nc.dram_tensor("st", (NL * 4 * 2, D, 2 * D), fp8, kind="ExternalInput")
    bias_d = nc.dram_tensor("bias", (D, NL * 4), f32, kind="ExternalInput")
    out_d = nc.dram_tensor("mp_out", (D, CH * D), bf16, kind="ExternalOutput")

    with tile.TileContext(nc) as tc:
        with (
            tc.tile_pool(name="const", bufs=1) as constp,
            tc.tile_pool(name="state", bufs=1) as statep,
            tc.tile_pool(name="gate", bufs=2) as gatep,
            tc.tile_pool(name="psum", bufs=1, space="PSUM") as psump,
        ):
            # ---- constants ----
            stT = [constp.tile([D, 4 * 2 * 2 * D], fp8, tag=f"stT{l}",
                               name=f"stT{l}")
                   for l in range(NL)]
            bias = constp.tile([D, NL * 4], f32, tag="bias")

            # ---- persistent state ----
            c_l = [statep.tile([D, CH * D], bf16, tag=f"c{l}", name=f"c{l}") for l in range(NL)]
            mp = statep.tile([D, CH * D], bf16, tag="mp")

            I0 = [statep.tile([D, NF], fp8, tag=f"I0{p}", name=f"I0{p}") for p in range(2)]
            I1 = [statep.tile([D, NF], fp8, tag=f"I1{p}", name=f"I1{p}") for p in range(2)]

            def seg3(t):  # (p, s, c) view of an input tile
                return t[:].rearrange("p (s c) -> p s c", s=CH)

            def pairview(t):  # (p, two, s, j129) DoubleRow moving view
                return t[:].rearrange("p (s j two) -> p two s j", s=CH, two=2)

            # startup: density for step 0/1 first; constants spread over queues
            nc.sync.dma_start(seg3(I0[0])[:, :, 1:1 + D], dens_d[0])
            nc.sync.dma_start(bias[:], bias_d[:])
            qs = [nc.scalar, nc.sync]
            order = [(l * 4 + g) * 2 + pr
                     for l in range(NL) for g in (3, 0, 1, 2) for pr in range(2)]
            for n, i in enumerate(order):
                l, j = divmod(i, 8)
                qs[n % 2].dma_start(stT[l][:, j * 2 * D:(j + 1) * 2 * D],
                                    st_d[i])
            nc.sync.dma_start(seg3(I0[1])[:, :, 1:1 + D], dens_d[1])

            # warm the sigmoid/tanh spline tables while DMAs run
            warm = constp.tile([D, 2], f32, tag="warm")
            nc.scalar.activation(warm[:, 0:1], bias[:, 0:1], AF.Sigmoid)
            nc.scalar.activation(warm[:, 1:2], bias[:, 0:1], AF.Tanh)

            for l in range(NL):
                nc.vector.memset(c_l[l][:], 0.0)
            nc.vector.memset(mp[:], -1e30)
            for t_ in I0 + I1:
                v = seg3(t_)
                nc.gpsimd.memset(v[:, :, 0:1].bitcast(i8), 0)
                nc.gpsimd.memset(v[:, :, SEG - 1:SEG].bitcast(i8), 0)
            nc.gpsimd.memset(seg3(I0[0])[:, :, 129:129 + D].bitcast(i8), 0)  # h1_{-1}
            nc.gpsimd.memset(seg3(I1[0])[:, :, 129:129 + D].bitcast(i8), 0)  # h2_{-1}

            GTAG = ["pf", "pi", "po", "pc"]
            AW = [OG0_ACT_CH * D, OG1_ACT_CH * D]   # og cols on ScalarE
            QW = [CH * D - w for w in AW]            # og cols on the approx

            def emit_back(l, t, og_parts):
                """Deferred tail of a layer-step: th = tanh(c) plus the h
                routing.  og_parts = (vo, og_act): the approx's v (cols
                [0, QW)) and the ScalarE og residual (cols [QW, 1024))."""
                vo, og_act = og_parts
                qw = QW[l]
                th = gatep.tile([D, CH * D], bf16, tag="th")
                nc.scalar.activation(th[:], c_l[l][:], AF.Tanh)
                h = gatep.tile([D, CH * D], bf16, tag="h")
                # h = (vo + 0.5) * th  |  og_act * th on the residual cols
                if qw:
                    nc.vector.scalar_tensor_tensor(
                        h[:, :qw], vo[:, :qw], 0.5, th[:, :qw],
                        op0=ALU.add, op1=ALU.mult)
                if qw < CH * D:
                    nc.vector.tensor_mul(h[:, qw:], og_act[:, qw:], th[:, qw:])
                if l == 0:
                    if t + 1 < L:
                        h1dst = seg3(I0[(t + 1) % 2])[:, :, 129:129 + D]
                        nc.gpsimd.tensor_copy(h1dst, h[:])   # bf16 -> fp8
                        nc.sync.dma_start(seg3(I1[t % 2])[:, :, 1:1 + D],
                                          h1dst)
                    else:
                        nc.gpsimd.tensor_copy(
                            seg3(I1[t % 2])[:, :, 1:1 + D], h[:])
                else:
                    if t + 1 < L:
                        nc.gpsimd.tensor_copy(
                            seg3(I1[(t + 1) % 2])[:, :, 129:129 + D], h[:])
                    nc.vector.tensor_tensor(mp[:], mp[:], h[:], op=ALU.max)

            def emit_P(l, t, pending=None):
                """Front of a layer-step: matmul burst -> cs/f/i activations
                + og quadratic approx -> cell update.  Returns a thunk for
                the deferred tail."""
                inp = I0[t % 2] if l == 0 else I1[t % 2]
                pv = pairview(inp)
                ps = {g: psump.tile([D, CH * D], f32, tag=GTAG[g], name=GTAG[g])
                      for g in range(4)}
                for g in (3, 0, 1, 2):
                    for half in range(2):
                        for pr in range(2):  # dh pairs (0,1) and (2,3)
                            idx = g * 2 + pr
                            lhsT = stT[l][:, idx * 2 * D:(idx + 1) * 2 * D] \
                                .rearrange("p (two m) -> p two m", two=2)
                            rhs = pv[:, :, half * 4:(half + 1) * 4, pr:pr + D]
                            nc.tensor.matmul(
                                ps[g][:, half * 512:(half + 1) * 512],
                                lhsT, rhs,
                                start=(pr == 0), stop=(pr == 1),
                                perf_mode=DR,
                            )
                qw = QW[l]
                # --- activations (bias folded in), chain-critical order ---
                fg = gatep.tile([D, CH * D], bf16, tag="fg")
                ig = gatep.tile([D, CH * D], bf16, tag="ig")
                cs = gatep.tile([D, CH * D], bf16, tag="cs")
                t1 = gatep.tile([D, CH * D], bf16, tag="t1")
                t2 = gatep.tile([D, CH * D], bf16, tag="t2")
                og = gatep.tile([D, CH * D], bf16, tag="og")
                oa = gatep.tile([D, CH * D], bf16, tag="oa")
                zc = gatep.tile([D, CH * D], bf16, tag="zc")
                ab = gatep.tile([D, CH * D], bf16, tag="ab")
                w2 = gatep.tile([D, CH * D], bf16, tag="w2")
                vo = gatep.tile([D, CH * D], bf16, tag="vo")
                # cs in halves: half0 only needs the first 2 pc matmuls
                nc.scalar.activation(cs[:, 0:512], ps[3][:, 0:512], AF.Tanh,
                                     bias=bias[:, l * 4 + 3: l * 4 + 4])
                if pending is not None:
                    pending()  # previous chain's th + h routing
                nc.scalar.activation(cs[:, 512:], ps[3][:, 512:], AF.Tanh,
                                     bias=bias[:, l * 4 + 3: l * 4 + 4])
                nc.scalar.activation(fg[:], ps[0][:], AF.Sigmoid,
                                     bias=bias[:, l * 4 + 0: l * 4 + 1])
                nc.scalar.activation(ig[:], ps[1][:], AF.Sigmoid,
                                     bias=bias[:, l * 4 + 1: l * 4 + 2])
                if qw < CH * D:
                    # og residual on ScalarE: psum holds s*pre, scale undoes it
                    nc.scalar.activation(og[:, qw:], ps[2][:, qw:], AF.Sigmoid,
                                         bias=bias[:, l * 4 + 2: l * 4 + 3],
                                         scale=1.0 / S_OG)
                # --- cell update (all bf16 in SBUF: DVE 2x mode) ---
                nc.vector.tensor_mul(t1[:], fg[:], c_l[l][:])
                nc.vector.tensor_mul(t2[:], ig[:], cs[:])
                nc.vector.tensor_add(c_l[l][:], t1[:], t2[:])
                if qw:
                    # --- og quadratic sigmoid: psum holds s*pre_o; bias col
                    # holds s*b_o.  a = min(s*pre + s*b, 1); zc = max(a, -1);
                    # v = max(1 - |a|/2, 0.5) * zc  == 0.5*(2-|zc|)*zc
                    nc.vector.tensor_scalar(oa[:, :qw], ps[2][:, :qw],
                                            bias[:, l * 4 + 2: l * 4 + 3], 1.0,
                                            op0=ALU.add, op1=ALU.min)
                    nc.gpsimd.tensor_scalar_max(zc[:, :qw], oa[:, :qw], -1.0)
                    nc.vector.scalar_tensor_tensor(ab[:, :qw], oa[:, :qw],
                                                   -1.0, oa[:, :qw],
                                                   op0=ALU.mult, op1=ALU.max)
                    nc.gpsimd.tensor_scalar(w2[:, :qw], ab[:, :qw], -0.5, 1.0,
                                            op0=ALU.mult, op1=ALU.add)
                    nc.vector.scalar_tensor_tensor(vo[:, :qw], w2[:, :qw],
                                                   0.5, zc[:, :qw],
                                                   op0=ALU.max, op1=ALU.mult)
                return lambda: emit_back(l, t, (vo, og))

            # skewed schedule: PE alternates between the two layers' bursts
            pend = emit_P(0, 0)
            for t in range(1, L):
                if t + 1 < L:
                    nc.sync.dma_start(seg3(I0[(t + 1) % 2])[:, :, 1:1 + D],
                                      dens_d[t + 1])
                if t == 1:
                    pend()
                    pend = None
                p0 = emit_P(0, t, pending=pend)
                pend = emit_P(1, t - 1, pending=p0)
            pend = emit_P(1, L - 1, pending=pend)
            pend()

            nc.sync.dma_start(out_d[:], mp[:])

    nc.compile()
    return nc


def _prep_core_inputs(dens_all, st, bias_arr, core):
    """dens_all: (B, 2, L, D, D) fp8 densities (axis1: 0=q, 1=a)."""
    sl = slice(4 * core, 4 * core + 4)
    ch = np.concatenate([dens_all[sl, 0], dens_all[sl, 1]], axis=0)  # (8, L, D, D)
    dens = np.ascontiguousarray(ch.transpose(1, 2, 0, 3)).reshape(L, D, CH * D)
    return {"dens": dens, "st": st, "bias": bias_arr}


def kernel(q, a, embed, conv_w, conv_b, lin_w, lin_b):
    import ml_dtypes
    from concourse import bass_utils

    fp8 = ml_dtypes.float8_e4m3
    q = np.asarray(q); a = np.asarray(a)
    embed = np.asarray(embed, np.float32)
    conv_w = np.asarray(conv_w, np.float32)
    conv_b = np.asarray(conv_b, np.float32)
    lin_w = np.asarray(lin_w, np.float32)
    lin_b = np.asarray(lin_b, np.float32)

    # host: embedding gather + density (normalized outer products)
    idx = np.stack([q, a], axis=1).astype(np.int64)            # (B, 2, L)
    xe = embed[idx].astype(np.float64)                         # (B, 2, L, D)
    dot = np.sum(xe * xe, axis=-1, keepdims=True) + 1e-4
    xe_y = (xe / np.sqrt(dot)).astype(np.float32)
    dens_all = np.einsum('bslw,bslj->bslwj', xe_y, xe_y).astype(fp8)

    # host: Toeplitz band stationaries, DoubleRow pair-interleaved.
    # og-gate (g=2) weights are pre-scaled by S_OG for the quadratic sigmoid.
    st = np.zeros((NL * 4 * 2, D, 2 * D), np.float32)
    for l in range(NL):
        for g in range(4):
            W = conv_w[l, g, 0, 0]                             # (4, 3)
            if g == 2:
                W = W * S_OG
            for dh in range(4):
                Bm = sum(W[dh, dw] * np.eye(D, k=dw - 1) for dw in range(3))
                pr, pp = dh // 2, dh % 2
                st[(l * 4 + g) * 2 + pr, :, pp * D:(pp + 1) * D] = \
                    Bm.T.astype(np.float32)
    st = st.astype(fp8)
    bias_host = conv_b.copy().reshape(NL, 4)
    bias_host[:, 2] *= S_OG                                    # og bias scaled
    bias_arr = np.tile(bias_host.reshape(1, -1), (D, 1)).astype(np.float32)

    if "nc" not in _CACHE:
        _CACHE["nc"] = _build_nc()
    nc = _CACHE["nc"]

    in_maps = [_prep_core_inputs(dens_all, st, bias_arr, i) for i in range(NCORES)]
    _CACHE["in_maps"] = in_maps
    res = bass_utils.run_bass_kernel_spmd(nc, in_maps, core_ids=list(range(NCORES)))

    # host: unshard + final linear + log_softmax
    q_p = np.zeros((B, D * D), np.float32)
    a_p = np.zeros((B, D * D), np.float32)
    for i in range(NCORES):
        out = np.asarray(res.results[i]["mp_out"]).astype(np.float32)  # (D w, CH*D)
        for s in range(CH):
            mp_T = out[:, s * D:(s + 1) * D]                   # (w, j)
            flat = np.ascontiguousarray(mp_T.T).reshape(-1)    # j-major
            if s < 4:
                q_p[4 * i + s] = flat
            else:
                a_p[4 * i + s - 4] = flat
    qa = np.concatenate([q_p, a_p], axis=1)
    score = qa @ lin_w.T + lin_b
    m = score.max(axis=1, keepdims=True)
    ls = score - m
    lse = np.log(np.exp(ls).sum(axis=1, keepdims=True))
    return (ls - lse).astype(np.float32)


# revision 4
# speedup vs baseline: 2.5638x; 2.5638x over previous
"""Trainium2 Bass kernel for NnqlmCnnBasedLstm.

Math (per batch item, per input sequence q/a):
  xe = embed[idx]                      (L, D)       D = 128
  dens_t = outer(xe_t, xe_t)/(|xe_t|^2 + 1e-4)     (D, D), symmetric
  2-layer ConvLSTM over L=40 steps; each gate g:
    pre_g = conv2d([xt; h], W_g, stride=(2,1), pad=(1,1)) + b_g  on (2D, D) -> (D, D)
  c = sig(f)*c + ig*tanh(cc); h = og*tanh(c)
  out = max_t h2_t  -> flatten -> concat(q,a) -> linear(2) -> log_softmax

Device strategy (8 cores, data parallel over B=32 -> 4 items/core, each with a
q-chain and an a-chain = 8 chains/core):
  * State kept TRANSPOSED: tiles are (w partitions, j free).  Densities are
    symmetric, precomputed on HOST, and DMAed per step (DMA engines are idle).
  * conv: out_T[w, j] = sum_{dh,dw} W[dh,dw] * inp_T[w-1+dw, 2j-1+dh].
    For each dh this is a 3-diagonal Toeplitz band matrix (over w) applied on
    the TensorEngine.  fp8e4m3 + perf_mode=DoubleRow packs the (dh0,dh1) and
    (dh2,dh3) band pairs as two K-planes of one matmul, halving the matmul
    count; fp32 PSUM accumulates the two pair-matmuls per gate.  All 8 chains
    batched in the moving free dim (2 x 512-col halves per PSUM bank limit).
  * The two ConvLSTM layers are software-pipelined with a 1-step skew
    (emit P0(t) then P1(t-1)).
  * Engine balance (ScalarE/ACT was the 94%-busy bottleneck; GpSimd tensor
    ops measured ~10x slower than DVE on HW, so they are NOT used for bulk
    elementwise work): f/i/cs/th stay on ScalarE.  Part of the OG sigmoid
    load moves to VectorE as a 5-segment piecewise-linear approx
      sig(x) ~ 0.5 + a*(clamp(z,-1,1) + clamp(z,-d,d)),  z = s*(x+b)
    built only from clamp-type tensor_scalar ops (4x DVE mode) + one
    tensor_tensor add (2x mode).  OG conv weights/bias are pre-scaled by s
    on the host so PSUM already holds z - s*b.  max |err| ~ 0.03, which the
    LSTM absorbs (validated end-to-end: rel err stays ~5e-3).
  * h is produced in bf16 on VectorE; its fp8 copies for the next-step conv
    inputs are produced by GpSimd-initiated *cast DMAs* (software DGE) --
    ~1us of idle-Pool time each, with the byte conversion riding the DMA
    hardware instead of DVE.
  * Cell updates on VectorE in bf16 (2x mode); running max-pool on VectorE.
  * Embedding gather, final linear + log_softmax on host (tiny).
"""

import os
import sys

import numpy as np

for _p in ("/opt/trn_rl_repo", "/root/.axon_site/_ro/trn_rl_repo"):
    if os.path.isdir(_p) and _p not in sys.path:
        sys.path.insert(0, _p)

B, L, D, V, NL = 32, 40, 128, 32000, 2
NCORES = 8
CH = 8            # chains per core: 4 batch items x {q, a}
SEG = 2 * D + 2   # per-chain column span in the input tile: [0]=0, [1..128]=x, [129..256]=h, [257]=0
NF = CH * SEG

# 5-segment PWL sigmoid: sig(x) ~ 0.5 + A*(clamp(z,-1,1)+clamp(z,-PWL_D,PWL_D))
# with z = PWL_S*x; fitted minimax, max abs err 0.0299.
PWL_S = 0.2874
PWL_D = 0.4275
PWL_A = 0.5 / (1.0 + PWL_D)
# og chains computed with the PWL approx on DVE (per layer); rest on ScalarE
OG_PWL_CH = [0, 8]

_CACHE = {}


def _build_nc(L=L):
    import concourse.bass as bass
    import concourse.bacc as bacc
    import concourse.mybir as mybir
    from concourse import tile

    f32 = mybir.dt.float32
    bf16 = mybir.dt.bfloat16
    fp8 = mybir.dt.float8e4
    i8 = mybir.dt.int8
    AF = mybir.ActivationFunctionType
    ALU = mybir.AluOpType
    DR = mybir.MatmulPerfMode.DoubleRow

    nc = bacc.Bacc(None, target_bir_lowering=False)

    dens_d = nc.dram_tensor("dens", (L, D, CH * D), fp8, kind="ExternalInput")
    st_d = nc.dram_tensor("st", (NL * 4 * 2, D, 2 * D), fp8, kind="ExternalInput")
    bias_d = nc.dram_tensor("bias", (D, NL * 4), f32, kind="ExternalInput")
    out_d = nc.dram_tensor("mp_out", (D, CH * D), bf16, kind="ExternalOutput")

    with tile.TileContext(nc) as tc:
        with (
            tc.tile_pool(name="const", bufs=1) as constp,
            tc.tile_pool(name="state", bufs=1) as statep,
            tc.tile_pool(name="gate", bufs=2) as gatep,
            tc.tile_pool(name="psum", bufs=1, space="PSUM") as psump,
        ):
            # ---- constants ----
            stT = [constp.tile([D, 4 * 2 * 2 * D], fp8, tag=f"stT{l}",
                               name=f"stT{l}")
                   for l in range(NL)]
            bias = constp.tile([D, NL * 4], f32, tag="bias")

            # ---- persistent state ----
            c_l = [statep.tile([D, CH * D], bf16, tag=f"c{l}", name=f"c{l}") for l in range(NL)]
            mp = statep.tile([D, CH * D], bf16, tag="mp")

            I0 = [statep.tile([D, NF], fp8, tag=f"I0{p}", name=f"I0{p}") for p in range(2)]
            I1 = [statep.tile([D, NF], fp8, tag=f"I1{p}", name=f"I1{p}") for p in range(2)]

            def seg3(t):  # (p, s, c) view of an input tile
                return t[:].rearrange("p (s c) -> p s c", s=CH)

            def pairview(t):  # (p, two, s, j129) DoubleRow moving view
                return t[:].rearrange("p (s j two) -> p two s j", s=CH, two=2)

            # startup: density for step 0/1 first; constants spread over queues
            nc.sync.dma_start(seg3(I0[0])[:, :, 1:1 + D], dens_d[0])
            nc.sync.dma_start(bias[:], bias_d[:])
            qs = [nc.scalar, nc.sync]
            order = [(l * 4 + g) * 2 + pr
                     for l in range(NL) for g in (3, 0, 1, 2) for pr in range(2)]
            for n, i in enumerate(order):
                l, j = divmod(i, 8)
                qs[n % 2].dma_start(stT[l][:, j * 2 * D:(j + 1) * 2 * D],
                                    st_d[i])
            nc.sync.dma_start(seg3(I0[1])[:, :, 1:1 + D], dens_d[1])

            # warm the sigmoid/tanh spline tables while DMAs run
            warm = constp.tile([D, 2], f32, tag="warm")
            nc.scalar.activation(warm[:, 0:1], bias[:, 0:1], AF.Sigmoid)
            nc.scalar.activation(warm[:, 1:2], bias[:, 0:1], AF.Tanh)

            for l in range(NL):
                nc.vector.memset(c_l[l][:], 0.0)
            nc.vector.memset(mp[:], -1e30)
            for t_ in I0 + I1:
                v = seg3(t_)
                nc.gpsimd.memset(v[:, :, 0:1].bitcast(i8), 0)
                nc.gpsimd.memset(v[:, :, SEG - 1:SEG].bitcast(i8), 0)
            nc.gpsimd.memset(seg3(I0[0])[:, :, 129:129 + D].bitcast(i8), 0)  # h1_{-1}
            nc.gpsimd.memset(seg3(I1[0])[:, :, 129:129 + D].bitcast(i8), 0)  # h2_{-1}

            GTAG = ["pf", "pi", "po", "pc"]
            QW = [c * D for c in OG_PWL_CH]          # og cols on the PWL path

            def emit_back(l, t, og_parts):
                """Deferred tail of a layer-step: th = tanh(c), h = og*th,
                then fp8 cast-DMAs of h into the next conv-input tiles."""
                q2, og_act = og_parts
                qw = QW[l]
                th = gatep.tile([D, CH * D], bf16, tag="th")
                nc.scalar.activation(th[:], c_l[l][:], AF.Tanh)
                h = gatep.tile([D, CH * D], bf16, tag="h")
                if qw:
                    nc.vector.tensor_mul(h[:, :qw], q2[:, :qw], th[:, :qw])
                if qw < CH * D:
                    nc.vector.tensor_mul(h[:, qw:], og_act[:, qw:], th[:, qw:])
                if l == 0:
                    # h1 -> fp8: conv-input of L0(t+1) and x-input of L1(t)
                    if t + 1 < L:
                        nc.gpsimd.dma_start(
                            seg3(I0[(t + 1) % 2])[:, :, 129:129 + D], h[:])
                    nc.gpsimd.dma_start(
                        seg3(I1[t % 2])[:, :, 1:1 + D], h[:])
                else:
                    if t + 1 < L:
                        nc.gpsimd.dma_start(
                            seg3(I1[(t + 1) % 2])[:, :, 129:129 + D], h[:])
                    nc.vector.tensor_tensor(mp[:], mp[:], h[:], op=ALU.max)

            def emit_P(l, t, pending=None):
                """Front of a layer-step: matmul burst -> cs/f/i (+og
                residual) activations + og PWL -> cell update."""
                inp = I0[t % 2] if l == 0 else I1[t % 2]
                pv = pairview(inp)
                ps = {g: psump.tile([D, CH * D], f32, tag=GTAG[g], name=GTAG[g])
                      for g in range(4)}
                for g in (3, 0, 1, 2):
                    for half in range(2):
                        for pr in range(2):  # dh pairs (0,1) and (2,3)
                            idx = g * 2 + pr
                            lhsT = stT[l][:, idx * 2 * D:(idx + 1) * 2 * D] \
                                .rearrange("p (two m) -> p two m", two=2)
                            rhs = pv[:, :, half * 4:(half + 1) * 4, pr:pr + D]
                            nc.tensor.matmul(
                                ps[g][:, half * 512:(half + 1) * 512],
                                lhsT, rhs,
                                start=(pr == 0), stop=(pr == 1),
                                perf_mode=DR,
                            )
                qw = QW[l]
                fg = gatep.tile([D, CH * D], bf16, tag="fg")
                ig = gatep.tile([D, CH * D], bf16, tag="ig")
                cs = gatep.tile([D, CH * D], bf16, tag="cs")
                t1 = gatep.tile([D, CH * D], bf16, tag="t1")
                t2 = gatep.tile([D, CH * D], bf16, tag="t2")
                og = gatep.tile([D, CH * D], bf16, tag="og")
                u1 = gatep.tile([D, CH * D], bf16, tag="u1")
                u1b = gatep.tile([D, CH * D], bf16, tag="u1b")
                u2 = gatep.tile([D, CH * D], bf16, tag="u2")
                qs_ = gatep.tile([D, CH * D], bf16, tag="qs")
                q2 = gatep.tile([D, CH * D], bf16, tag="q2")
                # cs in halves: half0 only needs the first 2 pc matmuls
                nc.scalar.activation(cs[:, 0:512], ps[3][:, 0:512], AF.Tanh,
                                     bias=bias[:, l * 4 + 3: l * 4 + 4])
                if pending is not None:
                    pending()  # previous chain's th + h routing
                nc.scalar.activation(cs[:, 512:], ps[3][:, 512:], AF.Tanh,
                                     bias=bias[:, l * 4 + 3: l * 4 + 4])
                nc.scalar.activation(fg[:], ps[0][:], AF.Sigmoid,
                                     bias=bias[:, l * 4 + 0: l * 4 + 1])
                nc.scalar.activation(ig[:], ps[1][:], AF.Sigmoid,
                                     bias=bias[:, l * 4 + 1: l * 4 + 2])
                if qw < CH * D:
                    # og residual on ScalarE: psum holds s*pre, scale undoes
                    nc.scalar.activation(og[:, qw:], ps[2][:, qw:], AF.Sigmoid,
                                         bias=bias[:, l * 4 + 2: l * 4 + 3],
                                         scale=1.0 / PWL_S)
                # --- cell update (all bf16 in SBUF: DVE 2x mode) ---
                nc.vector.tensor_mul(t1[:], fg[:], c_l[l][:])
                nc.vector.tensor_mul(t2[:], ig[:], cs[:])
                nc.vector.tensor_add(c_l[l][:], t1[:], t2[:])
                if qw:
                    # og PWL: psum holds z - s*b (weights pre-scaled); bias
                    # col holds s*b.  u1 = min(z, 1); u1b = max(u1, -1);
                    # u2 = clamp(u1, -d, d); q2 = A*(u1b+u2) + 0.5
                    nc.vector.tensor_scalar(u1[:, :qw], ps[2][:, :qw],
                                            bias[:, l * 4 + 2: l * 4 + 3], 1.0,
                                            op0=ALU.add, op1=ALU.min)
                    nc.vector.tensor_scalar_max(u1b[:, :qw], u1[:, :qw], -1.0)
                    nc.vector.tensor_scalar(u2[:, :qw], u1[:, :qw],
                                            PWL_D, -PWL_D,
                                            op0=ALU.min, op1=ALU.max)
                    nc.vector.tensor_add(qs_[:, :qw], u1b[:, :qw], u2[:, :qw])
                    nc.vector.tensor_scalar(q2[:, :qw], qs_[:, :qw],
                                            PWL_A, 0.5,
                                            op0=ALU.mult, op1=ALU.add)
                return lambda: emit_back(l, t, (q2, og))

            # skewed schedule: PE alternates between the two layers' bursts
            pend = emit_P(0, 0)
            for t in range(1, L):
                if t + 1 < L:
                    nc.sync.dma_start(seg3(I0[(t + 1) % 2])[:, :, 1:1 + D],
                                      dens_d[t + 1])
                if t == 1:
                    pend()
                    pend = None
                p0 = emit_P(0, t, pending=pend)
                pend = emit_P(1, t - 1, pending=p0)
            pend = emit_P(1, L - 1, pending=pend)
            pend()

            nc.sync.dma_start(out_d[:], mp[:])

    nc.compile()
    return nc


def _prep_core_inputs(dens_all, st, bias_arr, core):
    """dens_all: (B, 2, L, D, D) fp8 densities (axis1: 0=q, 1=a)."""
    sl = slice(4 * core, 4 * core + 4)
    ch = np.concatenate([dens_all[sl, 0], dens_all[sl, 1]], axis=0)  # (8, L, D, D)
    dens = np.ascontiguousarray(ch.transpose(1, 2, 0, 3)).reshape(L, D, CH * D)
    return {"dens": dens, "st": st, "bias": bias_arr}


def kernel(q, a, embed, conv_w, conv_b, lin_w, lin_b):
    import ml_dtypes
    from concourse import bass_utils

    fp8 = ml_dtypes.float8_e4m3
    q = np.asarray(q); a = np.asarray(a)
    embed = np.asarray(embed, np.float32)
    conv_w = np.asarray(conv_w, np.float32)
    conv_b = np.asarray(conv_b, np.float32)
    lin_w = np.asarray(lin_w, np.float32)
    lin_b = np.asarray(lin_b, np.float32)

    # host: embedding gather + density (normalized outer products)
    idx = np.stack([q, a], axis=1).astype(np.int64)            # (B, 2, L)
    xe = embed[idx].astype(np.float64)                         # (B, 2, L, D)
    dot = np.sum(xe * xe, axis=-1, keepdims=True) + 1e-4
    xe_y = (xe / np.sqrt(dot)).astype(np.float32)
    dens_all = np.einsum('bslw,bslj->bslwj', xe_y, xe_y).astype(fp8)

    # host: Toeplitz band stationaries, DoubleRow pair-interleaved.
    # og-gate (g=2) weights are pre-scaled by PWL_S for the PWL sigmoid.
    st = np.zeros((NL * 4 * 2, D, 2 * D), np.float32)
    for l in range(NL):
        for g in range(4):
            W = conv_w[l, g, 0, 0]                             # (4, 3)
            if g == 2:
                W = W * PWL_S
            for dh in range(4):
                Bm = sum(W[dh, dw] * np.eye(D, k=dw - 1) for dw in range(3))
                pr, pp = dh // 2, dh % 2
                st[(l * 4 + g) * 2 + pr, :, pp * D:(pp + 1) * D] = \
                    Bm.T.astype(np.float32)
    st = st.astype(fp8)
    bias_host = conv_b.copy().reshape(NL, 4)
    bias_host[:, 2] *= PWL_S                                   # og bias scaled
    bias_arr = np.tile(bias_host.reshape(1, -1), (D, 1)).astype(np.float32)

    if "nc" not in _CACHE:
        _CACHE["nc"] = _build_nc()
    nc = _CACHE["nc"]

    in_maps = [_prep_core_inputs(dens_all, st, bias_arr, i) for i in range(NCORES)]
    _CACHE["in_maps"] = in_maps
    res = bass_utils.run_bass_kernel_spmd(nc, in_maps, core_ids=list(range(NCORES)))

    # host: unshard + final linear + log_softmax
    q_p = np.zeros((B, D * D), np.float32)
    a_p = np.zeros((B, D * D), np.float32)
    for i in range(NCORES):
        out = np.asarray(res.results[i]["mp_out"]).astype(np.float32)  # (D w, CH*D)
        for s in range(CH):
            mp_T = out[:, s * D:(s + 1) * D]                   # (w, j)
            flat = np.ascontiguousarray(mp_T.T).reshape(-1)    # j-major
            if s < 4:
                q_p[4 * i + s] = flat
            else:
                a_p[4 * i + s - 4] = flat
    qa = np.concatenate([q_p, a_p], axis=1)
    score = qa @ lin_w.T + lin_b
    m = score.max(axis=1, keepdims=True)
    ls = score - m
    lse = np.log(np.exp(ls).sum(axis=1, keepdims=True))
    return (ls - lse).astype(np.float32)


# revision 6
# speedup vs baseline: 2.9348x; 1.1447x over previous
"""Trainium2 Bass kernel for NnqlmCnnBasedLstm.

Math (per batch item, per input sequence q/a):
  xe = embed[idx]                      (L, D)       D = 128
  dens_t = outer(xe_t, xe_t)/(|xe_t|^2 + 1e-4)     (D, D), symmetric
  2-layer ConvLSTM over L=40 steps; each gate g:
    pre_g = conv2d([xt; h], W_g, stride=(2,1), pad=(1,1)) + b_g  on (2D, D) -> (D, D)
  c = sig(f)*c + ig*tanh(cc); h = og*tanh(c)
  out = max_t h2_t  -> flatten -> concat(q,a) -> linear(2) -> log_softmax

Device strategy (8 cores, data parallel over B=32 -> 4 items/core, each with a
q-chain and an a-chain = 8 chains/core):
  * State kept TRANSPOSED: tiles are (w partitions, j free).  Densities are
    symmetric, precomputed on HOST, and DMAed per step (DMA engines are idle).
  * conv: out_T[w, j] = sum_{dh,dw} W[dh,dw] * inp_T[w-1+dw, 2j-1+dh].
    For each dh this is a 3-diagonal Toeplitz band matrix (over w) applied on
    the TensorEngine.  fp8e4m3 + perf_mode=DoubleRow packs the (dh0,dh1) and
    (dh2,dh3) band pairs as two K-planes of one matmul, halving the matmul
    count; fp32 PSUM accumulates the two pair-matmuls per gate.  All 8 chains
    batched in the moving free dim (2 x 512-col halves per PSUM bank limit).
  * The two ConvLSTM layers are software-pipelined with a 1-step skew
    (emit P0(t) then P1(t-1)).
  * Engine balance (ScalarE/ACT was the 94%-busy bottleneck; GpSimd tensor
    ops measured ~10x slower than DVE on HW, so they are NOT used for bulk
    elementwise work): f/i/cs/th stay on ScalarE.  Part of the OG sigmoid
    load moves to VectorE as a 5-segment piecewise-linear approx
      sig(x) ~ 0.5 + a*(clamp(z,-1,1) + clamp(z,-d,d)),  z = s*(x+b)
    built only from clamp-type tensor_scalar ops (4x DVE mode) + one
    tensor_tensor add (2x mode).  OG conv weights/bias are pre-scaled by s
    on the host so PSUM already holds z - s*b.  max |err| ~ 0.03, which the
    LSTM absorbs (validated end-to-end: rel err stays ~5e-3).
  * h is produced in bf16 on VectorE; its fp8 copies for the next-step conv
    inputs are produced by GpSimd-initiated *cast DMAs* (software DGE) --
    ~1us of idle-Pool time each, with the byte conversion riding the DMA
    hardware instead of DVE.
  * Cell updates on VectorE in bf16 (2x mode); running max-pool on VectorE.
  * Embedding gather, final linear + log_softmax on host (tiny).
"""

import os
import sys

import numpy as np

for _p in ("/opt/trn_rl_repo", "/root/.axon_site/_ro/trn_rl_repo"):
    if os.path.isdir(_p) and _p not in sys.path:
        sys.path.insert(0, _p)

B, L, D, V, NL = 32, 40, 128, 32000, 2
NCORES = 8
CH = 8            # chains per core: 4 batch items x {q, a}
SEG = 2 * D + 2   # per-chain column span in the input tile: [0]=0, [1..128]=x, [129..256]=h, [257]=0
NF = CH * SEG

# 5-segment PWL sigmoid: sig(x) ~ 0.5 + A*(clamp(z,-1,1)+clamp(z,-PWL_D,PWL_D))
# with z = PWL_S*x; fitted minimax, max abs err 0.0299.
PWL_S = 0.2874
PWL_D = 0.4275
PWL_A = 0.5 / (1.0 + PWL_D)
# og chains computed with the PWL approx on DVE (per layer); rest on ScalarE
OG_PWL_CH = [0, 8]

_CACHE = {}


def _build_nc(L=L):
    import concourse.bass as bass
    import concourse.bacc as bacc
    import concourse.mybir as mybir
    from concourse import tile

    f32 = mybir.dt.float32
    bf16 = mybir.dt.bfloat16
    fp8 = mybir.dt.float8e4
    i8 = mybir.dt.int8
    AF = mybir.ActivationFunctionType
    ALU = mybir.AluOpType
    DR = mybir.MatmulPerfMode.DoubleRow

    nc = bacc.Bacc(None, target_bir_lowering=False)

    dens_d = nc.dram_tensor("dens", (L, D, CH * D), fp8, kind="ExternalInput")
    st_d = nc.dram_tensor("st", (NL * 4 * 2, D, 2 * D), fp8, kind="ExternalInput")
    bias_d = nc.dram_tensor("bias", (D, NL * 4), f32, kind="ExternalInput")
    out_d = nc.dram_tensor("mp_out", (D, CH * D), bf16, kind="ExternalOutput")

    with tile.TileContext(nc) as tc:
        with (
            tc.tile_pool(name="const", bufs=1) as constp,
            tc.tile_pool(name="state", bufs=1) as statep,
            tc.tile_pool(name="gate", bufs=2) as gatep,
            tc.tile_pool(name="psum", bufs=1, space="PSUM") as psump,
        ):
            # ---- constants ----
            stT = [constp.tile([D, 4 * 2 * 2 * D], fp8, tag=f"stT{l}",
                               name=f"stT{l}")
                   for l in range(NL)]
            bias = constp.tile([D, NL * 4], f32, tag="bias")

            # ---- persistent state ----
            c_l = [statep.tile([D, CH * D], bf16, tag=f"c{l}", name=f"c{l}") for l in range(NL)]
            mp = statep.tile([D, CH * D], bf16, tag="mp")

            I0 = [statep.tile([D, NF], fp8, tag=f"I0{p}", name=f"I0{p}") for p in range(2)]
            I1 = [statep.tile([D, NF], fp8, tag=f"I1{p}", name=f"I1{p}") for p in range(2)]

            def seg3(t):  # (p, s, c) view of an input tile
                return t[:].rearrange("p (s c) -> p s c", s=CH)

            def pairview(t):  # (p, two, s, j129) DoubleRow moving view
                return t[:].rearrange("p (s j two) -> p two s j", s=CH, two=2)

            # startup: density for step 0/1 first; constants spread over queues
            nc.sync.dma_start(seg3(I0[0])[:, :, 1:1 + D], dens_d[0])
            nc.sync.dma_start(bias[:], bias_d[:])
            qs = [nc.scalar, nc.sync]
            order = [(l * 4 + g) * 2 + pr
                     for l in range(NL) for g in (3, 0, 1, 2) for pr in range(2)]
            for n, i in enumerate(order):
                l, j = divmod(i, 8)
                qs[n % 2].dma_start(stT[l][:, j * 2 * D:(j + 1) * 2 * D],
                                    st_d[i])
            nc.sync.dma_start(seg3(I0[1])[:, :, 1:1 + D], dens_d[1])

            # warm the sigmoid/tanh spline tables while DMAs run
            warm = constp.tile([D, 2], f32, tag="warm")
            nc.scalar.activation(warm[:, 0:1], bias[:, 0:1], AF.Sigmoid)
            nc.scalar.activation(warm[:, 1:2], bias[:, 0:1], AF.Tanh)

            for l in range(NL):
                nc.vector.memset(c_l[l][:], 0.0)
            nc.vector.memset(mp[:], -1e30)
            for t_ in I0 + I1:
                v = seg3(t_)
                nc.gpsimd.memset(v[:, :, 0:1].bitcast(i8), 0)
                nc.gpsimd.memset(v[:, :, SEG - 1:SEG].bitcast(i8), 0)
            nc.gpsimd.memset(seg3(I0[0])[:, :, 129:129 + D].bitcast(i8), 0)  # h1_{-1}
            nc.gpsimd.memset(seg3(I1[0])[:, :, 129:129 + D].bitcast(i8), 0)  # h2_{-1}

            GTAG = ["pf", "pi", "po", "pc"]
            QW = [c * D for c in OG_PWL_CH]          # og cols on the PWL path

            def emit_back(l, t, og_parts):
                """Deferred tail of a layer-step: th = tanh(c) plus the h
                routing (direct fp8 writes on VectorE, as latency matters:
                h feeds the next step's matmuls)."""
                q2, og_act = og_parts
                qw = QW[l]

                qseg = qw // D  # og-PWL chains (qw is a multiple of D)

                def hmul(dst3):
                    # dst3: (p, s, c) view; og source: q2 (PWL) / og_act (ACT)
                    q23 = q2[:].rearrange("p (s c) -> p s c", s=CH)
                    og3 = og_act[:].rearrange("p (s c) -> p s c", s=CH)
                    th3 = th[:].rearrange("p (s c) -> p s c", s=CH)
                    if qseg:
                        nc.vector.tensor_mul(dst3[:, :qseg], q23[:, :qseg],
                                             th3[:, :qseg])
                    if qseg < CH:
                        nc.vector.tensor_mul(dst3[:, qseg:], og3[:, qseg:],
                                             th3[:, qseg:])

                th = gatep.tile([D, CH * D], bf16, tag="th")
                nc.scalar.activation(th[:], c_l[l][:], AF.Tanh)
                if l == 0:
                    if t + 1 < L:
                        h1dst = seg3(I0[(t + 1) % 2])[:, :, 129:129 + D]
                        hmul(h1dst)
                        nc.sync.dma_start(seg3(I1[t % 2])[:, :, 1:1 + D],
                                          h1dst)
                    else:
                        hmul(seg3(I1[t % 2])[:, :, 1:1 + D])
                else:
                    if t + 1 < L:
                        hmul(seg3(I1[(t + 1) % 2])[:, :, 129:129 + D])
                    h2 = gatep.tile([D, CH * D], bf16, tag="h2")
                    hmul(h2[:].rearrange("p (s c) -> p s c", s=CH))
                    nc.vector.tensor_tensor(mp[:], mp[:], h2[:], op=ALU.max)

            def emit_P(l, t, pending=None):
                """Front of a layer-step: matmul burst -> cs/f/i (+og
                residual) activations + og PWL -> cell update."""
                inp = I0[t % 2] if l == 0 else I1[t % 2]
                pv = pairview(inp)
                ps = {g: psump.tile([D, CH * D], f32, tag=GTAG[g], name=GTAG[g])
                      for g in range(4)}
                for g in (3, 0, 1, 2):
                    for half in range(2):
                        for pr in range(2):  # dh pairs (0,1) and (2,3)
                            idx = g * 2 + pr
                            lhsT = stT[l][:, idx * 2 * D:(idx + 1) * 2 * D] \
                                .rearrange("p (two m) -> p two m", two=2)
                            rhs = pv[:, :, half * 4:(half + 1) * 4, pr:pr + D]
                            nc.tensor.matmul(
                                ps[g][:, half * 512:(half + 1) * 512],
                                lhsT, rhs,
                                start=(pr == 0), stop=(pr == 1),
                                perf_mode=DR,
                            )
                qw = QW[l]
                fg = gatep.tile([D, CH * D], bf16, tag="fg")
                ig = gatep.tile([D, CH * D], bf16, tag="ig")
                cs = gatep.tile([D, CH * D], bf16, tag="cs")
                t1 = gatep.tile([D, CH * D], bf16, tag="t1")
                t2 = gatep.tile([D, CH * D], bf16, tag="t2")
                og = gatep.tile([D, CH * D], bf16, tag="og")
                u1 = gatep.tile([D, CH * D], bf16, tag="u1")
                u1b = gatep.tile([D, CH * D], bf16, tag="u1b")
                u2 = gatep.tile([D, CH * D], bf16, tag="u2")
                qs_ = gatep.tile([D, CH * D], bf16, tag="qs")
                q2 = gatep.tile([D, CH * D], bf16, tag="q2")
                # cs in halves: half0 only needs the first 2 pc matmuls
                nc.scalar.activation(cs[:, 0:512], ps[3][:, 0:512], AF.Tanh,
                                     bias=bias[:, l * 4 + 3: l * 4 + 4])
                if pending is not None:
                    pending()  # previous chain's th + h routing
                nc.scalar.activation(cs[:, 512:], ps[3][:, 512:], AF.Tanh,
                                     bias=bias[:, l * 4 + 3: l * 4 + 4])
                nc.scalar.activation(fg[:], ps[0][:], AF.Sigmoid,
                                     bias=bias[:, l * 4 + 0: l * 4 + 1])
                nc.scalar.activation(ig[:], ps[1][:], AF.Sigmoid,
                                     bias=bias[:, l * 4 + 1: l * 4 + 2])
                if qw < CH * D:
                    # og residual on ScalarE: psum holds s*pre, scale undoes
                    nc.scalar.activation(og[:, qw:], ps[2][:, qw:], AF.Sigmoid,
                                         bias=bias[:, l * 4 + 2: l * 4 + 3],
                                         scale=1.0 / PWL_S)
                # --- cell update (all bf16 in SBUF: DVE 2x mode) ---
                nc.vector.tensor_mul(t1[:], fg[:], c_l[l][:])
                nc.vector.tensor_mul(t2[:], ig[:], cs[:])
                nc.vector.tensor_add(c_l[l][:], t1[:], t2[:])
                if qw:
                    # og PWL: psum holds z - s*b (weights pre-scaled); bias
                    # col holds s*b.  u1 = min(z, 1); u1b = max(u1, -1);
                    # u2 = clamp(u1, -d, d); q2 = A*(u1b+u2) + 0.5
                    nc.vector.tensor_scalar(u1[:, :qw], ps[2][:, :qw],
                                            bias[:, l * 4 + 2: l * 4 + 3], 1.0,
                                            op0=ALU.add, op1=ALU.min)
                    nc.vector.tensor_scalar_max(u1b[:, :qw], u1[:, :qw], -1.0)
                    nc.vector.tensor_scalar(u2[:, :qw], u1[:, :qw],
                                            PWL_D, -PWL_D,
                                            op0=ALU.min, op1=ALU.max)
                    nc.vector.tensor_add(qs_[:, :qw], u1b[:, :qw], u2[:, :qw])
                    nc.vector.tensor_scalar(q2[:, :qw], qs_[:, :qw],
                                            PWL_A, 0.5,
                                            op0=ALU.mult, op1=ALU.add)
                return lambda: emit_back(l, t, (q2, og))

            # skewed schedule: PE alternates between the two layers' bursts
            pend = emit_P(0, 0)
            for t in range(1, L):
                if t + 1 < L:
                    nc.sync.dma_start(seg3(I0[(t + 1) % 2])[:, :, 1:1 + D],
                                      dens_d[t + 1])
                if t == 1:
                    pend()
                    pend = None
                p0 = emit_P(0, t, pending=pend)
                pend = emit_P(1, t - 1, pending=p0)
            pend = emit_P(1, L - 1, pending=pend)
            pend()

            nc.sync.dma_start(out_d[:], mp[:])

    nc.compile()
    return nc


def _prep_core_inputs(dens_all, st, bias_arr, core):
    """dens_all: (B, 2, L, D, D) fp8 densities (axis1: 0=q, 1=a)."""
    sl = slice(4 * core, 4 * core + 4)
    ch = np.concatenate([dens_all[sl, 0], dens_all[sl, 1]], axis=0)  # (8, L, D, D)
    dens = np.ascontiguousarray(ch.transpose(1, 2, 0, 3)).reshape(L, D, CH * D)
    return {"dens": dens, "st": st, "bias": bias_arr}


def kernel(q, a, embed, conv_w, conv_b, lin_w, lin_b):
    import ml_dtypes
    from concourse import bass_utils

    fp8 = ml_dtypes.float8_e4m3
    q = np.asarray(q); a = np.asarray(a)
    embed = np.asarray(embed, np.float32)
    conv_w = np.asarray(conv_w, np.float32)
    conv_b = np.asarray(conv_b, np.float32)
    lin_w = np.asarray(lin_w, np.float32)
    lin_b = np.asarray(lin_b, np.float32)

    # host: embedding gather + density (normalized outer products)
    idx = np.stack([q, a], axis=1).astype(np.int64)            # (B, 2, L)
    xe = embed[idx].astype(np.float64)                         # (B, 2, L, D)
    dot = np.sum(xe * xe, axis=-1, keepdims=True) + 1e-4
    xe_y = (xe / np.sqrt(dot)).astype(np.float32)
    dens_all = np.einsum('bslw,bslj->bslwj', xe_y, xe_y).astype(fp8)

    # host: Toeplitz band stationaries, DoubleRow pair-interleaved.
    # og-gate (g=2) weights are pre-scaled by PWL_S for the PWL sigmoid.
    st = np.zeros((NL * 4 * 2, D, 2 * D), np.float32)
    for l in range(NL):
        for g in range(4):
            W = conv_w[l, g, 0, 0]                             # (4, 3)
            if g == 2:
                W = W * PWL_S
            for dh in range(4):
                Bm = sum(W[dh, dw] * np.eye(D, k=dw - 1) for dw in range(3))
                pr, pp = dh // 2, dh % 2
                st[(l * 4 + g) * 2 + pr, :, pp * D:(pp + 1) * D] = \
                    Bm.T.astype(np.float32)
    st = st.astype(fp8)
    bias_host = conv_b.copy().reshape(NL, 4)
    bias_host[:, 2] *= PWL_S                                   # og bias scaled
    bias_arr = np.tile(bias_host.reshape(1, -1), (D, 1)).astype(np.float32)

    if "nc" not in _CACHE:
        _CACHE["nc"] = _build_nc()
    nc = _CACHE["nc"]

    in_maps = [_prep_core_inputs(dens_all, st, bias_arr, i) for i in range(NCORES)]
    _CACHE["in_maps"] = in_maps
    res = bass_utils.run_bass_kernel_spmd(nc, in_maps, core_ids=list(range(NCORES)))

    # host: unshard + final linear + log_softmax
    q_p = np.zeros((B, D * D), np.float32)
    a_p = np.zeros((B, D * D), np.float32)
    for i in range(NCORES):
        out = np.asarray(res.results[i]["mp_out"]).astype(np.float32)  # (D w, CH*D)
        for s in range(CH):
            mp_T = out[:, s * D:(s + 1) * D]                   # (w, j)
            flat = np.ascontiguousarray(mp_T.T).reshape(-1)    # j-major
            if s < 4:
                q_p[4 * i + s] = flat
            else:
                a_p[4 * i + s - 4] = flat
    qa = np.concatenate([q_p, a_p], axis=1)
    score = qa @ lin_w.T + lin_b
    m = score.max(axis=1, keepdims=True)
    ls = score - m
    lse = np.log(np.exp(ls).sum(axis=1, keepdims=True))
    return (ls - lse).astype(np.float32)


# revision 9
# speedup vs baseline: 3.6289x; 1.2365x over previous
"""Trainium2 Bass kernel for NnqlmCnnBasedLstm.

Math (per batch item, per input sequence q/a):
  xe = embed[idx]                      (L, D)       D = 128
  dens_t = outer(xe_t, xe_t)/(|xe_t|^2 + 1e-4)     (D, D), symmetric
  2-layer ConvLSTM over L=40 steps; each gate g:
    pre_g = conv2d([xt; h], W_g, stride=(2,1), pad=(1,1)) + b_g  on (2D, D) -> (D, D)
  c = sig(f)*c + ig*tanh(cc); h = og*tanh(c)
  out = max_t h2_t  -> flatten -> concat(q,a) -> linear(2) -> log_softmax

Device strategy (8 cores, data parallel over B=32 -> 4 items/core, each with a
q-chain and an a-chain = 8 chains/core):
  * State kept TRANSPOSED: tiles are (w partitions, j free).  Densities are
    symmetric, precomputed on HOST, and DMAed per step (DMA engines are idle).
  * conv: out_T[w, j] = sum_{dh,dw} W[dh,dw] * inp_T[w-1+dw, 2j-1+dh].
    For each dh this is a 3-diagonal Toeplitz band matrix (over w) applied on
    the TensorEngine.  fp8e4m3 + perf_mode=DoubleRow packs the (dh0,dh1) and
    (dh2,dh3) band pairs as two K-planes of one matmul, halving the matmul
    count; fp32 PSUM accumulates the two pair-matmuls per gate.  All 8 chains
    batched in the moving free dim (2 x 512-col halves per PSUM bank limit).
  * The two ConvLSTM layers are software-pipelined with a 1-step skew
    (emit P0(t) then P1(t-1)).
  * Engine balance (ScalarE/ACT was the 94%-busy bottleneck; GpSimd tensor
    ops measured ~10x slower than DVE on HW, so they are NOT used for bulk
    elementwise work): f/i/cs/th stay on ScalarE.  Part of the OG sigmoid
    load moves to VectorE as a 5-segment piecewise-linear approx
      sig(x) ~ 0.5 + a*(clamp(z,-1,1) + clamp(z,-d,d)),  z = s*(x+b)
    built only from clamp-type tensor_scalar ops (4x DVE mode) + one
    tensor_tensor add (2x mode).  OG conv weights/bias are pre-scaled by s
    on the host so PSUM already holds z - s*b.  max |err| ~ 0.03, which the
    LSTM absorbs (validated end-to-end: rel err stays ~5e-3).
  * h is produced in bf16 on VectorE; its fp8 copies for the next-step conv
    inputs are produced by GpSimd-initiated *cast DMAs* (software DGE) --
    ~1us of idle-Pool time each, with the byte conversion riding the DMA
    hardware instead of DVE.
  * Cell updates on VectorE in bf16 (2x mode); running max-pool on VectorE.
  * Embedding gather, final linear + log_softmax on host (tiny).
"""

import os
import sys

import numpy as np

for _p in ("/opt/trn_rl_repo", "/root/.axon_site/_ro/trn_rl_repo"):
    if os.path.isdir(_p) and _p not in sys.path:
        sys.path.insert(0, _p)

B, L, D, V, NL = 32, 40, 128, 32000, 2
NCORES = 8
CH = 8            # chains per core: 4 batch items x {q, a}
SEG = 2 * D + 2   # per-chain column span in the input tile: [0]=0, [1..128]=x, [129..256]=h, [257]=0
NF = CH * SEG

# 5-segment PWL sigmoid: sig(x) ~ 0.5 + A*(clamp(z,-1,1)+clamp(z,-PWL_D,PWL_D))
# with z = PWL_S*x; fitted minimax, max abs err 0.0299.
PWL_S = 0.2874
PWL_D = 0.4275
PWL_A = 0.5 / (1.0 + PWL_D)
# og chains computed with the PWL approx on DVE (per layer); rest on ScalarE
OG_PWL_CH = [0, 0]

_CACHE = {}


def _build_nc(L=L):
    import concourse.bass as bass
    import concourse.bacc as bacc
    import concourse.mybir as mybir
    from concourse import tile

    f32 = mybir.dt.float32
    bf16 = mybir.dt.bfloat16
    fp8 = mybir.dt.float8e4
    i8 = mybir.dt.int8
    AF = mybir.ActivationFunctionType
    ALU = mybir.AluOpType
    DR = mybir.MatmulPerfMode.DoubleRow

    nc = bacc.Bacc(None, target_bir_lowering=False)

    dens_d = nc.dram_tensor("dens", (L, D, CH * D), fp8, kind="ExternalInput")
    st_d = nc.dram_tensor("st", (NL * 4 * 2, D, 2 * D), fp8, kind="ExternalInput")
    bias_d = nc.dram_tensor("bias", (D, NL * 4), f32, kind="ExternalInput")
    out_d = nc.dram_tensor("mp_out", (D, CH * D), bf16, kind="ExternalOutput")

    with tile.TileContext(nc) as tc:
        with (
            tc.tile_pool(name="const", bufs=1) as constp,
            tc.tile_pool(name="state", bufs=1) as statep,
            tc.tile_pool(name="gate", bufs=2) as gatep,
            tc.tile_pool(name="psum", bufs=1, space="PSUM") as psump,
        ):
            # ---- constants ----
            stT = [constp.tile([D, 4 * 2 * 2 * D], fp8, tag=f"stT{l}",
                               name=f"stT{l}")
                   for l in range(NL)]
            bias = constp.tile([D, NL * 4], f32, tag="bias")

            # ---- persistent state ----
            c_l = [statep.tile([D, CH * D], bf16, tag=f"c{l}", name=f"c{l}") for l in range(NL)]
            mp = statep.tile([D, CH * D], bf16, tag="mp")

            I0 = [statep.tile([D, NF], fp8, tag=f"I0{p}", name=f"I0{p}") for p in range(2)]
            I1 = [statep.tile([D, NF], fp8, tag=f"I1{p}", name=f"I1{p}") for p in range(2)]

            def seg3(t):  # (p, s, c) view of an input tile
                return t[:].rearrange("p (s c) -> p s c", s=CH)

            def pairview(t):  # (p, two, s, j129) DoubleRow moving view
                return t[:].rearrange("p (s j two) -> p two s j", s=CH, two=2)

            # startup: density for step 0/1 first; constants spread over queues
            nc.sync.dma_start(seg3(I0[0])[:, :, 1:1 + D], dens_d[0])
            nc.sync.dma_start(bias[:], bias_d[:])
            qs = [nc.scalar, nc.sync]
            order = [(l * 4 + g) * 2 + pr
                     for l in range(NL) for g in (3, 0, 1, 2) for pr in range(2)]
            for n, i in enumerate(order):
                l, j = divmod(i, 8)
                qs[n % 2].dma_start(stT[l][:, j * 2 * D:(j + 1) * 2 * D],
                                    st_d[i])
            nc.sync.dma_start(seg3(I0[1])[:, :, 1:1 + D], dens_d[1])

            # warm the sigmoid/tanh spline tables while DMAs run (input is
            # the warm tile itself -- garbage values are fine, this only
            # pulls the table set in without waiting on any DMA)
            warm = constp.tile([D, 2], f32, tag="warm")
            nc.scalar.activation(warm[:, 0:1], warm[:, 1:2], AF.Sigmoid)
            nc.scalar.activation(warm[:, 1:2], warm[:, 0:1], AF.Tanh)

            for l in range(NL):
                nc.vector.memset(c_l[l][:], 0.0)
            nc.vector.memset(mp[:], -1e30)
            for t_ in I0 + I1:
                v = seg3(t_)
                nc.gpsimd.memset(v[:, :, 0:1].bitcast(i8), 0)
                nc.gpsimd.memset(v[:, :, SEG - 1:SEG].bitcast(i8), 0)
            nc.gpsimd.memset(seg3(I0[0])[:, :, 129:129 + D].bitcast(i8), 0)  # h1_{-1}
            nc.gpsimd.memset(seg3(I1[0])[:, :, 129:129 + D].bitcast(i8), 0)  # h2_{-1}

            GTAG = ["pf", "pi", "po", "pc"]
            QW = [c * D for c in OG_PWL_CH]          # og cols on the PWL path

            def emit_back(l, t, og_parts):
                """Deferred tail of a layer-step: th = tanh(c) plus the h
                routing (direct fp8 writes on VectorE, as latency matters:
                h feeds the next step's matmuls)."""
                q2, og_act = og_parts
                qw = QW[l]

                qseg = qw // D  # og-PWL chains (qw is a multiple of D)

                def hmul(dst3):
                    # dst3: (p, s, c) view; og source: q2 (PWL) / og_act (ACT)
                    q23 = q2[:].rearrange("p (s c) -> p s c", s=CH)
                    og3 = og_act[:].rearrange("p (s c) -> p s c", s=CH)
                    th3 = th[:].rearrange("p (s c) -> p s c", s=CH)
                    if qseg:
                        nc.vector.tensor_mul(dst3[:, :qseg], q23[:, :qseg],
                                             th3[:, :qseg])
                    if qseg < CH:
                        nc.vector.tensor_mul(dst3[:, qseg:], og3[:, qseg:],
                                             th3[:, qseg:])

                th = gatep.tile([D, CH * D], bf16, tag="th")
                nc.scalar.activation(th[:], c_l[l][:], AF.Tanh)
                if l == 0:
                    if t + 1 < L:
                        h1dst = seg3(I0[(t + 1) % 2])[:, :, 129:129 + D]
                        hmul(h1dst)
                        nc.sync.dma_start(seg3(I1[t % 2])[:, :, 1:1 + D],
                                          h1dst)
                    else:
                        hmul(seg3(I1[t % 2])[:, :, 1:1 + D])
                else:
                    if t + 1 < L:
                        hmul(seg3(I1[(t + 1) % 2])[:, :, 129:129 + D])
                    h2 = gatep.tile([D, CH * D], bf16, tag="h2")
                    hmul(h2[:].rearrange("p (s c) -> p s c", s=CH))
                    nc.vector.tensor_tensor(mp[:], mp[:], h2[:], op=ALU.max)

            def emit_P(l, t, pending=None):
                """Front of a layer-step: matmul burst -> cs/f/i (+og
                residual) activations + og PWL -> cell update."""
                inp = I0[t % 2] if l == 0 else I1[t % 2]
                pv = pairview(inp)
                ps = {g: psump.tile([D, CH * D], f32, tag=GTAG[g], name=GTAG[g])
                      for g in range(4)}
                for g in (3, 0, 1, 2):
                    for half in range(2):
                        for pr in range(2):  # dh pairs (0,1) and (2,3)
                            idx = g * 2 + pr
                            lhsT = stT[l][:, idx * 2 * D:(idx + 1) * 2 * D] \
                                .rearrange("p (two m) -> p two m", two=2)
                            rhs = pv[:, :, half * 4:(half + 1) * 4, pr:pr + D]
                            nc.tensor.matmul(
                                ps[g][:, half * 512:(half + 1) * 512],
                                lhsT, rhs,
                                start=(pr == 0), stop=(pr == 1),
                                perf_mode=DR,
                            )
                qw = QW[l]
                fg = gatep.tile([D, CH * D], bf16, tag="fg")
                ig = gatep.tile([D, CH * D], bf16, tag="ig")
                cs = gatep.tile([D, CH * D], bf16, tag="cs")
                t1 = gatep.tile([D, CH * D], bf16, tag="t1")
                t2 = gatep.tile([D, CH * D], bf16, tag="t2")
                og = gatep.tile([D, CH * D], bf16, tag="og")
                u1 = gatep.tile([D, CH * D], bf16, tag="u1")
                u1b = gatep.tile([D, CH * D], bf16, tag="u1b")
                u2 = gatep.tile([D, CH * D], bf16, tag="u2")
                qs_ = gatep.tile([D, CH * D], bf16, tag="qs")
                q2 = gatep.tile([D, CH * D], bf16, tag="q2")
                # cs as one wide instruction: ACT is the saturated engine,
                # so fewer/wider instructions beat an earlier start
                nc.scalar.activation(cs[:], ps[3][:], AF.Tanh,
                                     bias=bias[:, l * 4 + 3: l * 4 + 4])
                if pending is not None:
                    pending()  # previous chain's th + h routing
                nc.scalar.activation(fg[:], ps[0][:], AF.Sigmoid,
                                     bias=bias[:, l * 4 + 0: l * 4 + 1])
                nc.scalar.activation(ig[:], ps[1][:], AF.Sigmoid,
                                     bias=bias[:, l * 4 + 1: l * 4 + 2])
                if qw < CH * D:
                    # og residual on ScalarE: psum holds s*pre, scale undoes
                    nc.scalar.activation(og[:, qw:], ps[2][:, qw:], AF.Sigmoid,
                                         bias=bias[:, l * 4 + 2: l * 4 + 3],
                                         scale=1.0 / PWL_S)
                # --- cell update (all bf16 in SBUF: DVE 2x mode) ---
                nc.vector.tensor_mul(t1[:], fg[:], c_l[l][:])
                nc.vector.tensor_mul(t2[:], ig[:], cs[:])
                nc.vector.tensor_add(c_l[l][:], t1[:], t2[:])
                if qw:
                    # og PWL: psum holds z - s*b (weights pre-scaled); bias
                    # col holds s*b.  u1 = min(z, 1); u1b = max(u1, -1);
                    # u2 = clamp(u1, -d, d); q2 = A*(u1b+u2) + 0.5
                    nc.vector.tensor_scalar(u1[:, :qw], ps[2][:, :qw],
                                            bias[:, l * 4 + 2: l * 4 + 3], 1.0,
                                            op0=ALU.add, op1=ALU.min)
                    nc.vector.tensor_scalar_max(u1b[:, :qw], u1[:, :qw], -1.0)
                    nc.vector.tensor_scalar(u2[:, :qw], u1[:, :qw],
                                            PWL_D, -PWL_D,
                                            op0=ALU.min, op1=ALU.max)
                    nc.vector.tensor_add(qs_[:, :qw], u1b[:, :qw], u2[:, :qw])
                    nc.vector.tensor_scalar(q2[:, :qw], qs_[:, :qw],
                                            PWL_A, 0.5,
                                            op0=ALU.mult, op1=ALU.add)
                return lambda: emit_back(l, t, (q2, og))

            # skewed schedule: PE alternates between the two layers' bursts
            pend = emit_P(0, 0)
            for t in range(1, L):
                if t + 1 < L:
                    nc.sync.dma_start(seg3(I0[(t + 1) % 2])[:, :, 1:1 + D],
                                      dens_d[t + 1])
                if t == 1:
                    pend()
                    pend = None
                p0 = emit_P(0, t, pending=pend)
                pend = emit_P(1, t - 1, pending=p0)
            pend = emit_P(1, L - 1, pending=pend)
            pend()

            nc.sync.dma_start(out_d[:], mp[:])

    nc.compile()
    return nc


def _prep_core_inputs(dens_all, st, bias_arr, core):
    """dens_all: (B, 2, L, D, D) fp8 densities (axis1: 0=q, 1=a)."""
    sl = slice(4 * core, 4 * core + 4)
    ch = np.concatenate([dens_all[sl, 0], dens_all[sl, 1]], axis=0)  # (8, L, D, D)
    dens = np.ascontiguousarray(ch.transpose(1, 2, 0, 3)).reshape(L, D, CH * D)
    return {"dens": dens, "st": st, "bias": bias_arr}


def kernel(q, a, embed, conv_w, conv_b, lin_w, lin_b):
    import ml_dtypes
    from concourse import bass_utils

    fp8 = ml_dtypes.float8_e4m3
    q = np.asarray(q); a = np.asarray(a)
    embed = np.asarray(embed, np.float32)
    conv_w = np.asarray(conv_w, np.float32)
    conv_b = np.asarray(conv_b, np.float32)
    lin_w = np.asarray(lin_w, np.float32)
    lin_b = np.asarray(lin_b, np.float32)

    # host: embedding gather + density (normalized outer products)
    idx = np.stack([q, a], axis=1).astype(np.int64)            # (B, 2, L)
    xe = embed[idx].astype(np.float64)                         # (B, 2, L, D)
    dot = np.sum(xe * xe, axis=-1, keepdims=True) + 1e-4
    xe_y = (xe / np.sqrt(dot)).astype(np.float32)
    dens_all = np.einsum('bslw,bslj->bslwj', xe_y, xe_y).astype(fp8)

    # host: Toeplitz band stationaries, DoubleRow pair-interleaved.
    # og-gate (g=2) weights are pre-scaled by PWL_S for the PWL sigmoid.
    st = np.zeros((NL * 4 * 2, D, 2 * D), np.float32)
    for l in range(NL):
        for g in range(4):
            W = conv_w[l, g, 0, 0]                             # (4, 3)
            if g == 2:
                W = W * PWL_S
            for dh in range(4):
                Bm = sum(W[dh, dw] * np.eye(D, k=dw - 1) for dw in range(3))
                pr, pp = dh // 2, dh % 2
                st[(l * 4 + g) * 2 + pr, :, pp * D:(pp + 1) * D] = \
                    Bm.T.astype(np.float32)
    st = st.astype(fp8)
    bias_host = conv_b.copy().reshape(NL, 4)
    bias_host[:, 2] *= PWL_S                                   # og bias scaled
    bias_arr = np.tile(bias_host.reshape(1, -1), (D, 1)).astype(np.float32)

    if "nc" not in _CACHE:
        _CACHE["nc"] = _build_nc()
    nc = _CACHE["nc"]

    in_maps = [_prep_core_inputs(dens_all, st, bias_arr, i) for i in range(NCORES)]
    _CACHE["in_maps"] = in_maps
    res = bass_utils.run_bass_kernel_spmd(nc, in_maps, core_ids=list(range(NCORES)))

    # host: unshard + final linear + log_softmax
    q_p = np.zeros((B, D * D), np.float32)
    a_p = np.zeros((B, D * D), np.float32)
    for i in range(NCORES):
        out = np.asarray(res.results[i]["mp_out"]).astype(np.float32)  # (D w, CH*D)
        for s in range(CH):
            mp_T = out[:, s * D:(s + 1) * D]                   # (w, j)
            flat = np.ascontiguousarray(mp_T.T).reshape(-1)    # j-major
            if s < 4:
                q_p[4 * i + s] = flat
            else:
                a_p[4 * i + s - 4] = flat
    qa = np.concatenate([q_p, a_p], axis=1)
    score = qa @ lin_w.T + lin_b
    m = score.max(axis=1, keepdims=True)
    ls = score - m
    lse = np.log(np.exp(ls).sum(axis=1, keepdims=True))
    return (ls - lse).astype(np.float32)
